# revision 1
# baseline (speedup 1.0000x reference)
"""Cross-attention (b=1, n=2048, dim=1024, 16 heads x 64) on 8 TRN2 NeuronCores.

Strategy:
- Tensor-parallel over heads: core k computes heads (2k, 2k+1) end to end and a
  partial output projection; host sums the 8 partials (the Wo all-reduce).
- Mask compaction on host: the padded mask pm gates both rows and columns of
  the attention matrix. Masked ROWS get uniform attention = (mean v) @ Wo,
  computed exactly on host; masked COLUMNS contribute exp(-inf)=0. So the
  device only computes attention over the C0 unmasked positions (padded to a
  multiple of 128), roughly halving all n^2 work.
- fp16 matmul datapath (fp32 accumulation in PSUM): ~2^-11 input rounding,
  1 cycle/row on the PE, DMA-transposable, fast weight load. Measured
  end-to-end relative error vs the fp32 reference is a few 1e-4.
- Softmax without max-subtraction (scores are O(1) by construction: scale is
  1/sqrt(dim) with unit-variance projections), column-pad masking folded into
  the Exp activation's per-partition bias, denominator via an all-ones
  matmul column-packed next to the P@V matmul, reciprocal as exp(-log(den))
  on ScalarE (same activation table set as the softmax Exp).
"""
import numpy as np

N_CORES = 8
HEADS = 16
DH = 64  # head dim
DIM = 1024
HPC = HEADS // N_CORES  # heads per core = 2

_cache = {}


def _build(C, JB, chunks):
    """Build + schedule the per-core Bass program for padded length C."""
    import concourse.mybir as mybir
    import concourse.tile as tile
    from concourse import bacc
    from concourse.masks import make_identity

    F32 = mybir.dt.float32
    F16 = mybir.dt.float16
    EXP = mybir.ActivationFunctionType.Exp
    LOG = mybir.ActivationFunctionType.Ln
    scale = DIM ** -0.5
    CB = DIM // 128  # contraction blocks for projections (8)

    nc = bacc.Bacc("TRN2", target_bir_lowering=False, debug=False)

    x_d = nc.dram_tensor("xh", [DIM, C], F16, kind="ExternalInput").ap()
    m_d = nc.dram_tensor("mh", [DIM, C], F16, kind="ExternalInput").ap()
    wq_d = nc.dram_tensor("wq", [DIM, 128], F16, kind="ExternalInput").ap()
    wk_d = nc.dram_tensor("wk", [DIM, 128], F16, kind="ExternalInput").ap()
    wv_d = nc.dram_tensor("wv", [DIM, 128], F16, kind="ExternalInput").ap()
    wo_d = nc.dram_tensor("wo", [128, DIM], F16, kind="ExternalInput").ap()
    jb_d = nc.dram_tensor("jbias", [128, JB], F32, kind="ExternalInput").ap()
    out_d = nc.dram_tensor("out", [C, DIM], F32, kind="ExternalOutput").ap()

    with tile.TileContext(nc) as tc:
        with (
            tc.tile_pool(name="persist", bufs=1) as pp,
            tc.tile_pool(name="outstage", bufs=3) as outp,
        ):
            # ---- persistent tiles ----
            # activations arrive pre-transposed from the host: plain DMA loads
            xT = pp.tile([128, CB, C], F16)
            mT = pp.tile([128, CB, C], F16)
            qT = pp.tile([128, C], F16)  # [d(2 heads), i]
            kT = pp.tile([128, C], F16)
            v1 = pp.tile([128, JB, 128], F16)  # v natural [j-in-block, jb, d]
            onesw = pp.tile([128, DH], F16)  # all-ones lhsT for den matmuls
            wo_sb = pp.tile([128, DIM], F16)  # Wo rows (both heads)
            wq_sb = pp.tile([128, CB, 128], F16)
            wk_sb = pp.tile([128, CB, 128], F16)
            wv_sb = pp.tile([128, CB, 128], F16)
            jbias = pp.tile([128, JB], F32)
            ON = pp.tile([128, C], F16)  # normalized attn out^T (both heads)

            nc.vector.memset(onesw[:], 1.0)
            for w_sb, d_ap in ((wq_sb, wq_d), (wk_sb, wk_d), (wv_sb, wv_d)):
                nc.gpsimd.dma_start(
                    w_sb[:], d_ap.rearrange("(cb p) d -> p cb d", p=128)
                )
            nc.gpsimd.dma_start(jbias[:], jb_d)
            nc.gpsimd.dma_start(wo_sb[:], wo_d)

            # ---- phase A+B: transposed loads interleaved with projections ----
            with tc.tile_pool(name="psB", bufs=1, space="PSUM") as psB:
                pq = [
                    psB.tile([128, 512], F32, name=f"pq{i}", tag=f"pq{i}")
                    for i in range(len(chunks))
                ]
                pk = [
                    psB.tile([128, 512], F32, name=f"pk{i}", tag=f"pk{i}")
                    for i in range(len(chunks))
                ]
                xr = x_d.rearrange("(cb p) i -> p cb i", p=128)
                mr = m_d.rearrange("(cb p) i -> p cb i", p=128)
                for g in range(0, CB, 2):
                    eng = nc.sync if (g // 2) % 2 == 0 else nc.scalar
                    eng.dma_start(xT[:, g : g + 2, :], xr[:, g : g + 2, :])
                    eng = nc.scalar if (g // 2) % 2 == 0 else nc.sync
                    eng.dma_start(mT[:, g : g + 2, :], mr[:, g : g + 2, :])
                for cb in range(CB):
                    for ci, (i0, cw) in enumerate(chunks):
                        nc.tensor.matmul(
                            pq[ci][:, :cw],
                            wq_sb[:, cb, :],
                            xT[:, cb, i0 : i0 + cw],
                            start=(cb == 0),
                            stop=(cb == CB - 1),
                        )
                        nc.tensor.matmul(
                            pk[ci][:, :cw],
                            wk_sb[:, cb, :],
                            mT[:, cb, i0 : i0 + cw],
                            start=(cb == 0),
                            stop=(cb == CB - 1),
                        )
                for ci, (i0, cw) in enumerate(chunks):
                    nc.vector.tensor_copy(qT[:, i0 : i0 + cw], pq[ci][:, :cw])
                    nc.vector.tensor_copy(kT[:, i0 : i0 + cw], pk[ci][:, :cw])

            # ---- phase B2: v natural: v[j, d] = sum_c m^T[c, j] Wv[c, d] ----
            with tc.tile_pool(name="psV", bufs=2, space="PSUM") as psV:
                for jb in range(JB):
                    pv = psV.tile([128, 128], F32, tag="pv")
                    for cb in range(CB):
                        nc.tensor.matmul(
                            pv[:],
                            mT[:, cb, jb * 128 : (jb + 1) * 128],
                            wv_sb[:, cb, :],
                            start=(cb == 0),
                            stop=(cb == CB - 1),
                        )
                    nc.vector.tensor_copy(v1[:, jb, :], pv[:])

            # ---- phase C: attention ----
            with (
                tc.tile_pool(name="ptp", bufs=2) as ptp,
                tc.tile_pool(name="nrm", bufs=3) as nrm,
                tc.tile_pool(name="psS", bufs=2, space="PSUM") as psS,
                tc.tile_pool(name="psO", bufs=1, space="PSUM") as psO,
                tc.tile_pool(name="psE", bufs=2, space="PSUM") as psE,
            ):
                for i0, cw in chunks:
                    PT = ptp.tile([128, JB, HPC, 512], F16, tag="PT")
                    for jb in range(JB):
                        sps = psS.tile([128, HPC, 512], F32, tag="S")
                        for h in range(HPC):
                            nc.tensor.matmul(
                                sps[:, h, :cw],
                                kT[h * DH : (h + 1) * DH, jb * 128 : (jb + 1) * 128],
                                qT[h * DH : (h + 1) * DH, i0 : i0 + cw],
                                start=True,
                                stop=True,
                            )
                        with nc.allow_low_precision(reason="softmax weights fp16"):
                            nc.scalar.activation(
                                PT[:, jb, :, :cw],
                                sps[:, :, :cw],
                                EXP,
                                bias=jbias[:, jb : jb + 1],
                                scale=scale,
                            )
                    # O^T and den, both heads col-packed into one PSUM tile each
                    ops = psO.tile([128, 512], F32, tag="O")
                    dps = psO.tile([128, 512], F32, tag="den")
                    for jb in range(JB):
                        for h in range(HPC):
                            nc.tensor.matmul(
                                ops[h * DH : (h + 1) * DH, :cw],
                                v1[:, jb, h * DH : (h + 1) * DH],
                                PT[:, jb, h, :cw],
                                start=(jb == 0),
                                stop=(jb == JB - 1),
                                tile_position=(0, h * DH),
                            )
                            nc.tensor.matmul(
                                dps[h * DH : (h + 1) * DH, :cw],
                                onesw[:],
                                PT[:, jb, h, :cw],
                                start=(jb == 0),
                                stop=(jb == JB - 1),
                                tile_position=(0, h * DH),
                            )
                    # normalize + output projection, pipelined per 128-row slice:
                    # recd = 1/den (per head half, already partition-aligned),
                    # ON = O * recd fused from PSUM, then Wo matmul
                    for isub in range(i0 // 128, (i0 + cw) // 128):
                        lo = isub * 128 - i0
                        recd = nrm.tile([128, 128], F32, tag="recd")
                        nc.vector.reciprocal(recd[:], dps[:, lo : lo + 128])
                        with nc.allow_low_precision(reason="attn out fp16"):
                            nc.vector.tensor_mul(
                                ON[:, isub * 128 : (isub + 1) * 128],
                                ops[:, lo : lo + 128],
                                recd[:],
                            )
                        ob = outp.tile([128, DIM], F32, tag="ob")
                        for eb in range(DIM // 512):
                            dp = psE.tile([128, 512], F32, tag="dout")
                            nc.tensor.matmul(
                                dp[:],
                                ON[:, isub * 128 : (isub + 1) * 128],
                                wo_sb[:, eb * 512 : (eb + 1) * 512],
                                start=True,
                                stop=True,
                            )
                            if eb % 2 == 0:
                                nc.vector.tensor_copy(
                                    ob[:, eb * 512 : (eb + 1) * 512], dp[:]
                                )
                            else:
                                nc.scalar.copy(ob[:, eb * 512 : (eb + 1) * 512], dp[:])
                        eng = nc.sync if isub % 2 == 0 else nc.scalar
                        eng.dma_start(out_d[isub * 128 : (isub + 1) * 128, :], ob[:])


    nc.compile()
    return nc


def _get_program(C, JB, chunks):
    key = (C, JB, tuple(chunks))
    if key not in _cache:
        _cache[key] = _build(C, JB, chunks)
    return _cache[key]


def kernel(x, m, mask, Wq, Wk, Wv, Wo, bo, _trace=False, _bass_results=None):
    from concourse.bass_utils import run_bass_kernel_spmd

    x = np.asarray(x)
    m = np.asarray(m)
    mask = np.asarray(mask)
    Wq, Wk, Wv, Wo, bo = (np.asarray(a, np.float32) for a in (Wq, Wk, Wv, Wo, bo))
    b, n, dim = x.shape
    assert (b, dim) == (1, DIM)

    pm = np.concatenate([np.array([True]), mask[0]])  # [n]
    sel = np.nonzero(pm)[0]
    C0 = len(sel)
    C = max(((C0 + 127) // 128) * 128, 256)
    JB = C // 128
    chunks = []
    i0 = 0
    while i0 < C:
        cw = min(512, C - i0)
        chunks.append((i0, cw))
        i0 += cw

    x_c = np.zeros((C, DIM), np.float16)
    x_c[:C0] = x[0][sel]
    m_c = np.zeros((C, DIM), np.float16)
    m_c[:C0] = m[0][sel]
    x_t = np.ascontiguousarray(x_c.T)  # [DIM, C]
    m_t = np.ascontiguousarray(m_c.T)

    jbias = np.zeros(C, np.float32)
    jbias[C0:] = -1e30
    jbias_t = np.ascontiguousarray(jbias.reshape(JB, 128).T)  # [128, JB]

    nc = _get_program(C, JB, chunks)

    in_maps = []
    for c in range(N_CORES):
        h0 = c * HPC * DH  # 128*c
        in_maps.append(
            {
                "xh": x_t,
                "mh": m_t,
                "wq": np.ascontiguousarray(Wq[:, h0 : h0 + 128]).astype(np.float16),
                "wk": np.ascontiguousarray(Wk[:, h0 : h0 + 128]).astype(np.float16),
                "wv": np.ascontiguousarray(Wv[:, h0 : h0 + 128]).astype(np.float16),
                "wo": np.ascontiguousarray(Wo[h0 : h0 + 128, :]).astype(np.float16),
                "jbias": jbias_t,
            }
        )

    res = run_bass_kernel_spmd(
        nc, in_maps, core_ids=list(range(N_CORES)), trace=_trace
    )
    if _bass_results is not None:
        _bass_results.append(res)

    acc = np.sum(
        np.stack([r["out"][:C0] for r in res.results]), axis=0, dtype=np.float64
    )

    # host-side: masked rows get uniform attention over ALL positions
    mv = m[0].astype(np.float64).mean(axis=0)  # mean over all j of m
    mv_out = (mv @ Wv.astype(np.float64)) @ Wo.astype(np.float64)  # [dim]

    out = np.empty((n, DIM), np.float64)
    out[sel] = acc
    out[~pm] = mv_out
    out += bo.astype(np.float64)
    return out[None].astype(np.float32)

